# revision 18
# baseline (speedup 1.0000x reference)
"""Fused pairwise-MLP kernel for Trainium2 (8 NeuronCores, SPMD data-parallel).

Computes log_q[i, j] = W3 @ gelu(W2 @ gelu(a[j] + b[i] + b1) + b2) + b3
with a = z1 @ W1a.T, b = z2 @ W1b.T  (W1 = [W1a | W1b]), N=1024, H=EMB=128.

Sharding: rows of i (z2) split across 8 cores, z1 + weights replicated
(host-side sharding; no collectives).

gelu1 engine tiers (host permutes i-rows by descending |x|-range so each
tier gets the rows its approximation can afford; permutation undone on
output gather):
  - ACT_SLOTS (8 widest): exact ACT gelu.
  - deg-6 positions: Vector-engine custom op (8 ALU stages, 1x) evaluating
    x' + ((x'^2+ct2)x'^2+ct3)x'^2 = SQB*2*gelu(x); W2 stationary pre-scaled.
  - QABS positions [72,120) (48 narrowest): a hand-written 2x_1P custom op
    (4 ALU ops: |x|, +c, ^2, +x -> 2 elems/lane/cycle) evaluating
    (beta|x|+c)^2 + beta*x ~ gelu/s_W, fed by a per-slot fp16 pre-bias
    tensor_scalar (4x mode).  The op is formally two-source (in1 = in0) so
    the perf-mode handler caps detection at 2x_1P (OneSrc would let the
    engine reach the unimplemented 4x slot and hang).  The quadratic's
    constant term is absorbed into a corrected gelu2 bias b2q.

gelu2 runs on ACT (PSUM input, bias port) as 2048-wide pairs + a 1024
single per 3-slot PSUM v-ring period, EXCEPT positions [108,120) which run
on the Vector engine (deg-6 even poly straight from PSUM, h2' = 2*gelu;
halved at evac via a per-partition scale vector).  W3 matmuls are emitted
3+ slots late and batched so the in-order PE stream never head-of-line
blocks the v-ring round-trip.

The W3 dot uses 32 zero-padded stationary variants (w3 at column k) so
slot s = 32g+k lands at PSUM partition s of a single [128,1024]
accumulation block.  Evac applies per-partition scale (1 or 0.5) and bias
(b3 + poly-offset corrections) from host-supplied vectors.
"""

import numpy as np

import concourse.bacc as bacc
import concourse.bass as bass
import concourse.bass_isa as bass_isa
import concourse.tile as tile
import concourse.mybir as mybir
from concourse import bass_utils

import concourse.dve_ops as dve_ops
from concourse.dve_ops import DveOp, OPS
from concourse.dve_spec import (
    Spec, Src0, Src1, C0, C1, C2, C3, PageIdx, lower, _spill_c3_to_src1,
    Bin, AluOp, sq,
)
from concourse.dve_uop import (
    DveOpSpec, UopConfig, InpSel, OutSel, AluInp, DelayInp,
    Trigger, ENABLE, OutPath,
)


def _register(name, spec, subdim, op_cls=DveOp):
    if name in dve_ops._SUB_OPCODE_FOR_NAME:
        return next(o for o in OPS if o.name == name)
    row = dve_ops._CUSTOM_DVE_ROW_BASE + len(OPS)
    dve_ops._SUB_OPCODE_FOR_NAME[name] = row
    shas = {}
    for ver in ("v3", "v4"):
        try:
            s = DveOpSpec(name=name, opcode=row, uops=lower(spec, ver=ver),
                          rd1_en=True)
            shas[ver] = s.sha(ver)
        except Exception:
            pass
    op = op_cls(name, spec, subdim=subdim, uops_sha=shas)
    OPS.append(op)
    dve_ops.CUSTOM_DVE_SPECS[name] = spec
    return op


def _gelu1_single_spec():
    # x = in0 + s0; u = x*x; out = ((s1*u + imm2)*u + c3)*u + x
    x = Src0 + C0
    u = x * x
    t = ((C1 * u + C2) * u + C3) * u
    body = _spill_c3_to_src1(t + x)

    def ref(in0, in1, s0, s1, imm2):
        xx = in0.astype(np.float32) + s0
        uu = xx * xx
        return ((s1 * uu + imm2) * uu + in1) * uu + xx

    return Spec(body=body, reference=ref)


def _gelu1_pair_spec():
    # in0 = [P, 2, N]; bias steps via PageIdx(C0, C1); leading coef = 1
    pg = PageIdx(C0, C1)
    x = Src0 + pg
    u = x * x
    t = ((u + C2) * u + C3) * u
    body = _spill_c3_to_src1(t + x)

    def ref(in0, in1, s0, s1, imm2):
        x = in0.astype(np.float32)
        S = int(np.prod(x.shape[1:-1]))
        x3 = x.reshape((x.shape[0], S, x.shape[-1]))
        idx = s0[:, None] if isinstance(s0, np.ndarray) else s0
        s1v = s1[:, None] if isinstance(s1, np.ndarray) else s1
        bias = idx + np.arange(S, dtype=np.float32)[None, :, None] * s1v
        xx = x3 + bias
        uu = xx * xx
        c3v = in1 if not isinstance(in1, np.ndarray) else in1.reshape(-1, 1, 1)
        y = ((uu + imm2) * uu + c3v) * uu + xx
        return y.reshape(in0.shape)

    return Spec(body=body, reference=ref)


# --- QABS 2x op: y = (|x0| + c)^2 + x1, hand-written 2x_1P program ---------


def _qabs_spec():
    t = Bin(AluOp.ABSOLUTE_VALUE, Src0, Src0)
    body = sq(t + C0) + Src1

    def ref(in0, in1, s0, s1, imm2):
        x = in0.astype(np.float32)
        return (np.abs(x) + s0) ** 2 + in1.astype(np.float32)

    return Spec(body=body, reference=ref)


def _qabs_2x_uop():
    """2x_1P program: lo chain blocks 0-3, hi chain 4-7.
    inp0=SRC_0(x_lo), d0=SRC_0_HI, d1=CONST_0, d2=SRC_1, d3=SRC_1_HI."""
    u = UopConfig()
    u.enable_input(InpSel.SRC_0, 0)
    u.enable_input(InpSel.SRC_0_HI, 1)
    u.enable_input(InpSel.CONST_0, 2)
    u.enable_input(InpSel.SRC_1, 3)
    u.enable_input(InpSel.SRC_1_HI, 4)
    dp = u.datapath_config
    P = AluInp.PREV_ALU_OUT
    D0, D1, D2, D3 = (AluInp.PREV_DELAY_0, AluInp.PREV_DELAY_1,
                      AluInp.PREV_DELAY_2, AluInp.PREV_DELAY_3)
    dp[0].enable_alu(AluOp.ABSOLUTE_VALUE, P, P)
    dp[0].pass_through_delay(0, 1, 2, 3)
    dp[1].enable_alu(AluOp.ADD, P, D1)
    dp[1].pass_through_delay(0, 1, 2, 3)
    dp[2].enable_alu(AluOp.MULTIPLY, P, P)
    dp[2].pass_through_delay(0, 1, 2, 3)
    dp[3].enable_alu(AluOp.ADD, P, D2)
    dp[3].pass_through_delay(0, 1, 3)
    dp[4].enable_alu(AluOp.ABSOLUTE_VALUE, D0, D0)
    dp[4].pass_through_delay(1, 3)
    dp[4].enable_delay_from_src(DelayInp.PREV_ALU_OUT, 2)
    dp[5].enable_alu(AluOp.ADD, P, D1)
    dp[5].pass_through_delay(2, 3)
    dp[6].enable_alu(AluOp.MULTIPLY, P, P)
    dp[6].pass_through_delay(2, 3)
    dp[7].enable_alu(AluOp.ADD, P, D3)
    dp[7].pass_through_delay(2)
    u.out[OutPath.WR0_LO] = OutSel.DELAY_2
    u.out_enable[OutPath.WR0_LO] = ENABLE
    u.out[OutPath.WR0_HI] = OutSel.ALU_OUT
    u.out_enable[OutPath.WR0_HI] = ENABLE
    u.require_inp0 = 1
    u.require_inp1 = 1
    u.trigger = (Trigger.SRC_TENSOR_DONE, Trigger.NONE, Trigger.NONE)
    u.next_uop = (0, 0, 0)
    return u


class DveOp2x(DveOp):
    """DveOp whose compiled spec carries the hand-written 2x program."""

    def compile(self, ver):
        key = (self.name, ver)
        cache = dve_ops._COMPILE_CACHE
        if (r := cache.get(key)) is not None:
            return r
        uops_1x = lower(self.spec, ver=ver)
        assert len(uops_1x) == 1
        result = DveOpSpec(
            name=self.name,
            opcode=dve_ops.get_dve_sub_opcode(self.name),
            uops=uops_1x,
            uops_2x=[_qabs_2x_uop()],
            perf_max=1,
            rd1_en=True,
        )
        cache[key] = result
        return result


GELU1_OP = _register("GELU1_EVEN6_ANT", _gelu1_single_spec(), subdim=False)
GELU1P_OP = _register("GELU1_PAIR_ANT", _gelu1_pair_spec(), subdim=True)
QABS_OP = _register("QABS_GELU_2X_ANT", _qabs_spec(), subdim=False,
                    op_cls=DveOp2x)


def _emit_qabs_2x(vec, op, *, out, in0, in1, s0):
    """Mirror of Vector._custom_dve with perf_max=1 (two-src op)."""
    nc_b = vec.bass
    if op.name not in nc_b.m.ant_custom_dve_ops:
        nc_b.m.ant_custom_dve_ops = sorted({*nc_b.m.ant_custom_dve_ops, op.name})
    shape = bass_isa.CustomDveShape.TTSS
    isa_opcode = nc_b.isa.Opcode[
        f"NEURON_ISA_TPB_OPCODE_CUSTOM_DVE_ANT_{shape.slot()}"
    ].value
    ins = [vec.lower_ap(in0, for_isa=True, opt=True),
           vec.lower_ap(in1, for_isa=True, opt=True),
           mybir.ImmediateValue(dtype=mybir.dt.float32, value=float(s0)),
           mybir.ImmediateValue(dtype=mybir.dt.float32, value=0.0)]
    outs = [vec.lower_ap(out, for_isa=True, opt=True)]
    return vec.add_instruction(
        mybir.InstCustomDveAnt(
            name=nc_b.get_next_instruction_name(),
            op_name=op.name,
            rd1_en=True,
            subdim=0,
            imm2=0.0,
            shape=shape,
            row=dve_ops.get_dve_sub_opcode(op.name),
            isa_opcode=isa_opcode,
            ins=ins,
            outs=outs,
            perf_max=1,
        )
    )


# deg-6 even fit of x*erf(x/sqrt(2)) on |x| <= 3.67 (density-weighted,
# x ~ N(0, 0.586)): coefs of u, u^2, u^3
CC = (0.7720335236204651, -0.09365603610221726, 0.00457457167839083)
BETA = CC[2] ** 0.4               # leading-coef normalization
SQB = float(np.sqrt(BETA))
CT2 = float(CC[1] / BETA ** 1.5)  # u'^2 coef after normalization
CT3 = float(CC[0] / SQB)          # u'^1 coef

N = 1024
EMB = 128
HID = 128
NCORES = 8
SH = N // NCORES  # i-slots per core
F32 = mybir.dt.float32
FP16 = mybir.dt.float16
GELU = mybir.ActivationFunctionType.Gelu
COPY = mybir.ActivationFunctionType.Copy

# gelu1 tier layout (all boundaries divisible by 3 so no gelu2 pair
# straddles a bias/engine boundary)
ACT_SLOTS = (6, 15, 24, 33, 42, 51, 60, 69)   # widest rows, exact gelu
QL, QH = 72, 120                               # qabs 2x tier (narrowest)
G2D_LO, G2D_HI = 108, 120                      # gelu2 on DVE (h2' = 2*gelu)
DEG6_SLOTS = tuple(p for p in range(SH)
                   if p not in ACT_SLOTS and not (QL <= p < QH))

# qabs half-fit 0.5*t*erf(t/sqrt2) ~ A t^2 + B t + C on [0, 3.1]
# (density-weighted).  beta = 2A (input prescale), c = B, s_W = 0.25/A
# (W2 stationary scale), delta = C - B^2/(4A) -> b2q correction.
QA_A = 0.17234688
QA_B = 0.18103941
QA_C = -0.02535395
QA_BETA = 2.0 * QA_A
QA_C0 = QA_B
QA_SW = 0.25 / QA_A
QA_DELTA = QA_C - QA_B * QA_B / (4.0 * QA_A)

# deg-6 even fit of t*erf(t/sqrt(2)) ~ c1*u + c2*u^2 + c3*u^3 + c0 for
# the DVE gelu2 tail (u = t^2, t = y + b2q), density-weighted.
G2_CU = 0.77404693    # u coef    -> C3 (in1 tile)
G2_CU2 = -0.10035028  # u^2 coef  -> C2 (imm2)
G2_CU3 = 0.00622154   # u^3 coef  -> C1 (s1)
G2_D0 = 0.00230938    # offset    -> folded into b3adj on the host


def _build(b3val):
    nc = bacc.Bacc("TRN2", target_bir_lowering=False, debug=False)

    z1Tq_d = [
        nc.dram_tensor(f"z1Tq{q}", (EMB, 256), FP16, kind="ExternalInput")
        for q in range(4)
    ]
    z2T_d = nc.dram_tensor("z2T", (EMB, SH), F32, kind="ExternalInput")
    w1aT_d = nc.dram_tensor("w1aT", (EMB, HID), FP16, kind="ExternalInput")
    w1bT_d = nc.dram_tensor("w1bT", (EMB, HID), F32, kind="ExternalInput")
    w2T_d = nc.dram_tensor("w2T", (HID, HID), F32, kind="ExternalInput")
    w3v_d = nc.dram_tensor("w3v", (HID, 1024), F32, kind="ExternalInput")
    b1_d = nc.dram_tensor("b1", (HID,), F32, kind="ExternalInput")
    b2_d = nc.dram_tensor("b2", (HID,), F32, kind="ExternalInput")
    b2q_d = nc.dram_tensor("b2q", (HID,), F32, kind="ExternalInput")
    b3adj_d = nc.dram_tensor("b3adj", (HID,), F32, kind="ExternalInput")
    evsc_d = nc.dram_tensor("evsc", (HID,), F32, kind="ExternalInput")
    out_d = nc.dram_tensor("out", (SH, N), F32, kind="ExternalOutput")

    with tile.TileContext(nc) as tc:
        _body(tc, out_d, z1Tq_d, z2T_d, w1aT_d, w1bT_d, w2T_d, w3v_d,
              b1_d, b2_d, b2q_d, b3adj_d, evsc_d)

    nc.compile()
    return nc


def _body(tc, out_d, z1Tq_d, z2T_d, w1aT_d, w1bT_d, w2T_d, w3v_d,
          b1_d, b2_d, b2q_d, b3adj_d, evsc_d):
    nc = tc.nc
    with (
        tc.tile_pool(name="const", bufs=1) as const,
        tc.tile_pool(name="h1p", bufs=4) as h1p,
        tc.tile_pool(name="h2p", bufs=3) as h2p,
        tc.tile_pool(name="srows", bufs=1) as srows,
        tc.tile_pool(name="ringp", bufs=1, space="PSUM") as ringp,
    ):
        # ACT warms the gelu table as its very first instruction (no DMAs
        # ride the scalar queue at startup).
        tiny = const.tile([1, 1], F32)
        nc.vector.memset(tiny, 0.0)
        warm = const.tile([1, 1], F32)
        nc.scalar.activation(warm, tiny, GELU)

        # ---- input DMAs: z1T quarters lead the HWDGE queues ----
        z1T_sb = const.tile([128, N], FP16)
        for q, eng in enumerate((nc.sync, nc.scalar, nc.sync, nc.scalar)):
            eng.dma_start(out=z1T_sb[:, q * 256:(q + 1) * 256], in_=z1Tq_d[q].ap())
        w1aT_sb = const.tile([128, HID], FP16)
        nc.gpsimd.dma_start(out=w1aT_sb, in_=w1aT_d.ap())
        w1bT_sb = const.tile([128, HID], F32)
        nc.gpsimd.dma_start(out=w1bT_sb, in_=w1bT_d.ap())
        z2T_sb = const.tile([128, SH], F32)
        nc.sync.dma_start(out=z2T_sb, in_=z2T_d.ap())
        w2T_f = const.tile([128, HID], F32)
        nc.scalar.dma_start(out=w2T_f, in_=w2T_d.ap())
        w3v_f = const.tile([128, 1024], F32)
        nc.gpsimd.dma_start(out=w3v_f, in_=w3v_d.ap())
        b1_sb = const.tile([128, 1], F32)
        nc.gpsimd.dma_start(out=b1_sb, in_=b1_d.ap().rearrange("(p o) -> p o", o=1))
        b2_sb = const.tile([128, 1], F32)
        nc.gpsimd.dma_start(out=b2_sb, in_=b2_d.ap().rearrange("(p o) -> p o", o=1))
        b2q_sb = const.tile([128, 1], F32)
        nc.sync.dma_start(out=b2q_sb, in_=b2q_d.ap().rearrange("(p o) -> p o", o=1))
        b3adj_sb = const.tile([128, 1], F32)
        nc.sync.dma_start(out=b3adj_sb,
                          in_=b3adj_d.ap().rearrange("(p o) -> p o", o=1))
        evsc_sb = const.tile([128, 1], F32)
        nc.sync.dma_start(out=evsc_sb,
                          in_=evsc_d.ap().rearrange("(p o) -> p o", o=1))

        c3p_sb = const.tile([128, 1], F32)
        nc.gpsimd.memset(c3p_sb, CT3)
        c3g2_sb = const.tile([128, 1], F32)
        nc.gpsimd.memset(c3g2_sb, G2_CU)

        # fp16 stationaries (w2T on the startup-idle ACT, w3v on Pool)
        w2T_full = const.tile([128, HID], FP16)
        nc.scalar.activation(w2T_full, w2T_f, COPY, bias=0.0)
        w2T_half = const.tile([128, HID], FP16)  # x(0.5/SQB): h1' = SQB*2*gelu
        nc.scalar.activation(w2T_half, w2T_f, COPY, bias=0.0, scale=0.5 / SQB)
        w2T_q = const.tile([128, HID], FP16)     # x s_W for the qabs tier
        nc.scalar.activation(w2T_q, w2T_f, COPY, bias=0.0, scale=QA_SW)
        w3v_h = const.tile([128, 1024], FP16)
        nc.gpsimd.tensor_copy(w3v_h, w3v_f)

        # ---- PSUM: 3 v-slots + [128,1024] W3 accumulation block ----
        ring = ringp.tile([128, 4096], F32)
        VS = [ring[:, 0:1024], ring[:, 1024:2048], ring[:, 2048:3072]]
        w3blk = ring[:, 3072:4096]

        # ---- prologue: b_pp tiles, scaled duplicated a ----
        tpb = ring[:, 2048:2048 + SH]   # v-slot 2 region, freed before use
        nc.tensor.matmul(tpb, w1bT_sb, z2T_sb)
        b_pp_sc = const.tile([128, SH], F32)       # SQB*(b + b1)
        nc.vector.tensor_scalar(out=b_pp_sc, in0=tpb, scalar1=b1_sb[:, 0:1],
                                scalar2=SQB, op0=mybir.AluOpType.add,
                                op1=mybir.AluOpType.mult)
        b_pp = const.tile([128, SH], F32)          # b + b1 (ACT slots)
        nc.vector.tensor_scalar(out=b_pp, in0=tpb, scalar1=b1_sb[:, 0:1],
                                scalar2=None, op0=mybir.AluOpType.add)
        d_sc = const.tile([128, SH], F32)          # pair bias deltas
        nc.vector.tensor_tensor(out=d_sc[:, 0:SH - 1], in0=b_pp_sc[:, 1:SH],
                                in1=b_pp_sc[:, 0:SH - 1],
                                op=mybir.AluOpType.subtract)

        tpa = ring[:, 0:1024]
        for q in range(4):
            nc.tensor.matmul(tpa[:, q * 256:(q + 1) * 256], w1aT_sb,
                             z1T_sb[:, q * 256:(q + 1) * 256])
        a_dbl = const.tile([128, 2048], F32)       # SQB*a, twice
        nc.vector.tensor_scalar(out=a_dbl[:, 0:1024], in0=tpa, scalar1=SQB,
                                scalar2=None, op0=mybir.AluOpType.mult)
        nc.scalar.activation(a_dbl[:, 1024:2048], tpa, COPY, bias=0.0,
                             scale=SQB)

        # qabs-tier inputs (on Pool -- off the DVE/ACT critical paths;
        # not needed until position 72's batch, ~half-way into the run)
        a4 = const.tile([128, 1024], FP16)         # QA_BETA * a
        nc.gpsimd.tensor_scalar(out=a4, in0=a_dbl[:, 0:1024],
                                scalar1=QA_BETA / SQB, scalar2=None,
                                op0=mybir.AluOpType.mult)
        b4_sb = const.tile([128, SH], F32)         # QA_BETA * (b + b1)
        nc.gpsimd.tensor_scalar(out=b4_sb, in0=b_pp,
                                scalar1=QA_BETA, scalar2=None,
                                op0=mybir.AluOpType.mult)

        # ---- steady state ----
        srow = srows.tile([128, N], F32)
        h1map = {}

        def pump_g1(upto):
            s = pump_g1.next
            while s < min(upto, SH):
                if s in ACT_SLOTS:
                    h1 = h1p.tile([128, N], FP16, tag="h1s", name="h1s", bufs=3)
                    nc.scalar.activation(h1, a_dbl[:, 0:1024], GELU,
                                         bias=b_pp[:, s:s + 1], scale=1.0 / SQB)
                    h1map[s] = (h1, 0)
                    s += 1
                elif QL <= s < QH:
                    # 4-slot qabs batch: 4 fp16 pre-bias TS (4x) + one
                    # 2x_1P custom op over the packed [128,4096] tile
                    x4 = h1p.tile([128, 4096], FP16, tag="x4", name="x4",
                                  bufs=2)
                    for i in range(4):
                        nc.vector.tensor_scalar(
                            out=x4[:, i * 1024:(i + 1) * 1024], in0=a4,
                            scalar1=b4_sb[:, s + i:s + i + 1], scalar2=None,
                            op0=mybir.AluOpType.add)
                    h1q = h1p.tile([128, 4096], FP16, tag="h1q", name="h1q",
                                   bufs=3)
                    _emit_qabs_2x(nc.vector, QABS_OP, out=h1q, in0=x4,
                                  in1=x4, s0=QA_C0)
                    for i in range(4):
                        h1map[s + i] = (h1q, i * 1024)
                    s += 4
                elif s + 1 < SH and (s + 1) not in ACT_SLOTS and not (
                        QL <= s + 1 < QH):
                    h1 = h1p.tile([128, 2048], FP16, tag="h1d", name="h1d",
                                  bufs=6)
                    nc.vector._custom_dve(
                        GELU1P_OP,
                        out=h1[:, :].rearrange("p (s n) -> p s n", n=N),
                        in0=a_dbl[:, :].rearrange("p (s n) -> p s n", n=N),
                        in1=c3p_sb[:, 0:1],
                        s0=b_pp_sc[:, s:s + 1], s1=d_sc[:, s:s + 1], imm2=CT2)
                    h1map[s] = (h1, 0)
                    h1map[s + 1] = (h1, 1024)
                    s += 2
                else:
                    h1 = h1p.tile([128, N], FP16, tag="h1s", name="h1s", bufs=3)
                    nc.vector._custom_dve(
                        GELU1_OP, out=h1, in0=a_dbl[:, 0:1024],
                        in1=c3p_sb[:, 0:1],
                        s0=b_pp_sc[:, s:s + 1], s1=1.0, imm2=CT2)
                    h1map[s] = (h1, 0)
                    s += 1
            pump_g1.next = s

        pump_g1.next = 0

        def emit_w2(s):
            h1, off = h1map.pop(s)
            if s in ACT_SLOTS:
                w2 = w2T_full
            elif QL <= s < QH:
                w2 = w2T_q
            else:
                w2 = w2T_half
            vs = VS[s % 3]
            for h in range(2):
                nc.tensor.matmul(vs[:, h * 512:(h + 1) * 512], w2,
                                 h1[:, off + h * 512:off + (h + 1) * 512])

        def emit_w3(s, h2, off):
            g, k = divmod(s, 32)
            w3k = w3v_h[:, 32 * k:32 * k + 32]
            for h in range(2):
                nc.tensor.matmul(
                    w3blk[32 * g:32 * g + 32, h * 512:(h + 1) * 512],
                    w3k, h2[:, off + h * 512:off + (h + 1) * 512],
                    tile_position=(0, 32 * g),
                    start=(k == 0), stop=(k == 31), skip_group_check=True)

        h2q = []

        def g2_bias(s):
            return b2q_sb if QL <= s < QH else b2_sb

        def emit_g2_pair(s0, s1):
            h2 = h2p.tile([128, 2048], FP16, tag="h2", name="h2", bufs=5)
            nc.scalar.activation(h2, ring[:, (s0 % 3) * 1024:(s0 % 3) * 1024 + 2048],
                                 GELU, bias=g2_bias(s0)[:, 0:1])
            h2q.append((s0, h2, 0))
            h2q.append((s1, h2, 1024))

        def emit_g2_single(s):
            h2 = h2p.tile([128, 1024], FP16, tag="h2s", name="h2s", bufs=4)
            nc.scalar.activation(h2, VS[s % 3], GELU, bias=g2_bias(s)[:, 0:1])
            h2q.append((s, h2, 0))

        def emit_g2_dve(s0, width):
            # deg-6 poly straight from PSUM; h2' = 2*gelu (evac halves it).
            # b2q bias is identical across slots, so a flat pair is fine.
            tag = "h2" if width == 2048 else "h2s"
            h2 = h2p.tile([128, width], FP16, tag=tag, name=tag,
                          bufs=5 if width == 2048 else 4)
            in0 = ring[:, (s0 % 3) * 1024:(s0 % 3) * 1024 + width]
            nc.vector._custom_dve(
                GELU1_OP, out=h2, in0=in0, in1=c3g2_sb[:, 0:1],
                s0=b2q_sb[:, 0:1], s1=G2_CU3, imm2=G2_CU2)
            for i in range(width // 1024):
                h2q.append((s0 + i, h2, i * 1024))

        def evac(g):
            # g==2: groups 0-2 in one pass (partition count is free);
            # g==3: final group, lands in the drain tail.
            if g < 2:
                return
            lo, hi = (0, 96) if g == 2 else (96, 128)
            nc.vector.tensor_scalar(
                out=srow[lo:hi, :], in0=w3blk[lo:hi, :],
                scalar1=evsc_sb[lo:hi, 0:1], scalar2=b3adj_sb[lo:hi, 0:1],
                op0=mybir.AluOpType.mult, op1=mybir.AluOpType.add)
            nc.sync.dma_start(out=out_d.ap()[lo:hi, :], in_=srow[lo:hi, :])

        next_evac = 0
        w3_done = -1
        for s in range(SH):
            pump_g1(s + 6)
            emit_w2(s)
            r = s % 3
            if G2D_LO <= s < G2D_HI:
                if r == 1:
                    emit_g2_dve(s - 1, 2048)
                elif r == 2:
                    emit_g2_dve(s, 1024)
            elif r == 1:
                emit_g2_pair(s - 1, s)
            elif r == 2:
                emit_g2_single(s)
            # W3 batched after the W2 legs of the next pair (see docstring)
            if r == 1:
                while h2q and h2q[0][0] <= s - 3:
                    sl, h2, off = h2q.pop(0)
                    emit_w3(sl, h2, off)
                    w3_done = sl
            while w3_done >= 32 * next_evac + 31:
                evac(next_evac)
                next_evac += 1
        while h2q:
            sl, h2, off = h2q.pop(0)
            emit_w3(sl, h2, off)
            w3_done = sl
            while w3_done >= 32 * next_evac + 31:
                evac(next_evac)
                next_evac += 1


_NC_CACHE = {}


def make_in_maps(z1, z2, W1, b1, W2, b2, W3, b3):
    f = np.float32
    z1 = np.asarray(z1, dtype=f)
    z2 = np.asarray(z2, dtype=f)
    W1 = np.asarray(W1, dtype=f)
    b1 = np.ascontiguousarray(np.asarray(b1, dtype=f))
    W2 = np.asarray(W2, dtype=f)
    b2 = np.ascontiguousarray(np.asarray(b2, dtype=f))
    W3 = np.asarray(W3, dtype=f)
    b3 = np.ascontiguousarray(np.asarray(b3, dtype=f))

    z1T = np.ascontiguousarray(z1.T.astype(np.float16))
    z1Tq = {
        f"z1Tq{q}": np.ascontiguousarray(z1T[:, q * 256:(q + 1) * 256])
        for q in range(4)
    }
    w1aT = np.ascontiguousarray(W1[:, :EMB].T.astype(np.float16))
    w1bT = np.ascontiguousarray(W1[:, EMB:].T)
    w2T = np.ascontiguousarray(W2.T)
    w3v = np.zeros((HID, 1024), dtype=f)
    for k in range(32):
        w3v[:, 32 * k + k] = W3[0]

    # corrected gelu2 bias for qabs-tier slots (quadratic's constant term
    # flows through W2), and per-partition evac scale/bias vectors
    b2q = b2 + np.float32(QA_DELTA) * W2.sum(axis=1).astype(f)
    b3adj = np.full(HID, b3[0], dtype=f)
    b3adj[G2D_LO:G2D_HI] += 0.5 * G2_D0 * W3[0].sum()
    evsc = np.ones(HID, dtype=f)
    evsc[G2D_LO:G2D_HI] = 0.5

    # slot permutation: widest-range i's (per core) -> ACT_SLOTS, middle ->
    # deg-6 positions, narrowest -> qabs positions (the [108,120) sub-range
    # of which also runs its gelu2 as a DVE poly)
    a = z1 @ W1[:, :EMB].T            # host copy, scheduling only
    bb = z2 @ W1[:, EMB:].T + b1
    amin, amax = a.min(0), a.max(0)
    ximax = np.maximum(np.abs(amin[None, :] + bb), np.abs(amax[None, :] + bb)).max(1)

    maps, perms = [], []
    act_slots = list(ACT_SLOTS)
    deg6_slots = list(DEG6_SLOTS)
    qabs_slots = list(range(QL, QH))
    for c in range(NCORES):
        loc = ximax[c * SH:(c + 1) * SH]
        order = np.argsort(-loc)
        perm = np.empty(SH, dtype=np.int64)
        perm[act_slots] = order[:len(act_slots)]
        perm[deg6_slots] = order[len(act_slots):len(act_slots) + len(deg6_slots)]
        perm[qabs_slots] = order[len(act_slots) + len(deg6_slots):]
        perms.append(perm)
        z2c = z2[c * SH:(c + 1) * SH][perm]
        maps.append({
            **z1Tq,
            "z2T": np.ascontiguousarray(z2c.T),
            "w1aT": w1aT, "w1bT": w1bT, "w2T": w2T, "w3v": w3v,
            "b1": b1, "b2": b2, "b2q": b2q, "b3adj": b3adj, "evsc": evsc,
        })
    return maps, perms


def gather_out(results, perms):
    blocks = []
    for c in range(NCORES):
        o = results[c]["out"]
        inv = np.empty(SH, dtype=np.int64)
        inv[perms[c]] = np.arange(SH)
        blocks.append(o[inv])
    return np.concatenate(blocks, axis=0)


def kernel(z1, z2, W1, b1, W2, b2, W3, b3):
    b3v = float(np.asarray(b3).reshape(-1)[0])
    key = round(b3v, 9)
    if key not in _NC_CACHE:
        _NC_CACHE[key] = _build(b3v)
    nc = _NC_CACHE[key]

    in_maps, perms = make_in_maps(z1, z2, W1, b1, W2, b2, W3, b3)
    res = bass_utils.run_bass_kernel_spmd(nc, in_maps, core_ids=list(range(NCORES)))
    return gather_out(res.results, perms)


if __name__ == "__main__":
    rng = np.random.default_rng(0)
    s1 = 1.0 / np.sqrt(2 * EMB)
    s2 = 1.0 / np.sqrt(HID)
    ins = dict(
        z1=rng.standard_normal((N, EMB), dtype=np.float32),
        z2=rng.standard_normal((N, EMB), dtype=np.float32),
        W1=rng.uniform(-s1, s1, (HID, 2 * EMB)).astype(np.float32),
        b1=rng.uniform(-s1, s1, (HID,)).astype(np.float32),
        W2=rng.uniform(-s2, s2, (HID, HID)).astype(np.float32),
        b2=rng.uniform(-s2, s2, (HID,)).astype(np.float32),
        W3=rng.uniform(-s2, s2, (1, HID)).astype(np.float32),
        b3=rng.uniform(-s2, s2, (1,)).astype(np.float32),
    )
    out = kernel(**ins)
    print("out", out.shape, out.dtype, out[:2, :4])


# revision 19
# speedup vs baseline: 1.1363x; 1.1363x over previous
"""Fused pairwise-MLP kernel for Trainium2 (8 NeuronCores, SPMD data-parallel).

Computes log_q[i, j] = W3 @ gelu(W2 @ gelu(a[j] + b[i] + b1) + b2) + b3
with a = z1 @ W1a.T, b = z2 @ W1b.T  (W1 = [W1a | W1b]), N=1024, H=EMB=128.

Sharding: rows of i (z2) split across 8 cores, z1 + weights replicated
(host-side sharding; no collectives).

The two gelu passes (2 x 131072 128-partition columns per core) are the
arithmetic bottleneck: the ACT engine runs them at 1 elem/lane/cycle and
nothing else on the chip has a gelu table.  gelu1 therefore runs almost
entirely on the Vector engine as a runtime-registered custom DVE op that
evaluates, in one 8-ALU-stage pass over two slots (2048 cols, subdim
form, per-slot bias via PageIdx),
    y = x' + ((x'^2 + ct2)*x'^2 + ct3)*x'^2,   x' = sqrt(beta)*(a + bias)
which equals sqrt(beta)*2*gelu(x) for a beta-normalized deg-6 even fit
(the leading Horner coefficient is normalized to 1 so the three scalar
ports cover bias-step + two coefficients).  The W2 stationary for these
slots is pre-scaled by 0.5/sqrt(beta).  The NA widest-range slots per
core run on ACT's exact gelu (host permutes i-rows so they land on the
fixed ACT slot positions; permutation undone on output gather).

gelu2 runs entirely on ACT (PSUM input, b2 via the bias port) as
2048-wide pair instructions + a 1024 single per 3-slot PSUM v-ring
period.  W3 matmuls are emitted 3+ slots late and batched after the
next period's W2 legs so the in-order PE stream never head-of-line
blocks the v-ring round-trip (pair-gelu2 -> 2x W2 -> next pair).

The W3 dot uses 32 zero-padded stationary variants (w3 at column k) so
slot s = 32g+k lands at PSUM partition s of a single [128,1024]
accumulation block (tile_position group g, accumulating matmuls).  All
128 output rows are evacuated with 4 instructions (+b3) and 5 DMAs.
"""

import numpy as np

import concourse.bacc as bacc
import concourse.bass as bass
import concourse.tile as tile
import concourse.mybir as mybir
from concourse import bass_utils

import concourse.dve_ops as dve_ops
from concourse.dve_ops import DveOp, OPS
from concourse.dve_spec import (
    Spec, Src0, C0, C1, C2, C3, PageIdx, lower, _spill_c3_to_src1,
)
from concourse.dve_uop import DveOpSpec


def _register(name, spec, subdim):
    if name in dve_ops._SUB_OPCODE_FOR_NAME:
        return next(o for o in OPS if o.name == name)
    row = dve_ops._CUSTOM_DVE_ROW_BASE + len(OPS)
    dve_ops._SUB_OPCODE_FOR_NAME[name] = row
    shas = {}
    for ver in ("v3", "v4"):
        try:
            s = DveOpSpec(name=name, opcode=row, uops=lower(spec, ver=ver),
                          rd1_en=True)
            shas[ver] = s.sha(ver)
        except Exception:
            pass
    op = DveOp(name, spec, subdim=subdim, uops_sha=shas)
    OPS.append(op)
    dve_ops.CUSTOM_DVE_SPECS[name] = spec
    return op


def _gelu1_single_spec():
    # x = in0 + s0; u = x*x; out = ((s1*u + imm2)*u + c3)*u + x
    x = Src0 + C0
    u = x * x
    t = ((C1 * u + C2) * u + C3) * u
    body = _spill_c3_to_src1(t + x)

    def ref(in0, in1, s0, s1, imm2):
        xx = in0.astype(np.float32) + s0
        uu = xx * xx
        return ((s1 * uu + imm2) * uu + in1) * uu + xx

    return Spec(body=body, reference=ref)


def _gelu1_pair_spec():
    # in0 = [P, 2, N]; bias steps via PageIdx(C0, C1); leading coef = 1
    pg = PageIdx(C0, C1)
    x = Src0 + pg
    u = x * x
    t = ((u + C2) * u + C3) * u
    body = _spill_c3_to_src1(t + x)

    def ref(in0, in1, s0, s1, imm2):
        x = in0.astype(np.float32)
        S = int(np.prod(x.shape[1:-1]))
        x3 = x.reshape((x.shape[0], S, x.shape[-1]))
        idx = s0[:, None] if isinstance(s0, np.ndarray) else s0
        s1v = s1[:, None] if isinstance(s1, np.ndarray) else s1
        bias = idx + np.arange(S, dtype=np.float32)[None, :, None] * s1v
        xx = x3 + bias
        uu = xx * xx
        c3v = in1 if not isinstance(in1, np.ndarray) else in1.reshape(-1, 1, 1)
        y = ((uu + imm2) * uu + c3v) * uu + xx
        return y.reshape(in0.shape)

    return Spec(body=body, reference=ref)


GELU1_OP = _register("GELU1_EVEN6_ANT", _gelu1_single_spec(), subdim=False)
GELU1P_OP = _register("GELU1_PAIR_ANT", _gelu1_pair_spec(), subdim=True)

# deg-6 even fit of x*erf(x/sqrt(2)) on |x| <= 3.67 (density-weighted,
# x ~ N(0, 0.586)): coefs of u, u^2, u^3
CC = (0.7720335236204651, -0.09365603610221726, 0.00457457167839083)
BETA = CC[2] ** 0.4               # leading-coef normalization
SQB = float(np.sqrt(BETA))
CT2 = float(CC[1] / BETA ** 1.5)  # u'^2 coef after normalization
CT3 = float(CC[0] / SQB)          # u'^1 coef

N = 1024
EMB = 128
HID = 128
NCORES = 8
SH = N // NCORES  # i-slots per core
F32 = mybir.dt.float32
FP16 = mybir.dt.float16
GELU = mybir.ActivationFunctionType.Gelu
COPY = mybir.ActivationFunctionType.Copy

# slots whose gelu1 runs on ACT (exact); host routes widest-range i's
# here.  Gaps of 15 keep every DVE run even-length (pairable).
ACT_SLOTS = tuple(range(6, 112, 15))  # 8 slots: 6,21,...,111
DVE_SLOTS = tuple(s for s in range(SH) if s not in ACT_SLOTS)


def _build(b3val):
    nc = bacc.Bacc("TRN2", target_bir_lowering=False, debug=False)

    z1Tq_d = [
        nc.dram_tensor(f"z1Tq{q}", (EMB, 256), FP16, kind="ExternalInput")
        for q in range(4)
    ]
    z2T_d = nc.dram_tensor("z2T", (EMB, SH), F32, kind="ExternalInput")
    w1aT_d = nc.dram_tensor("w1aT", (EMB, HID), FP16, kind="ExternalInput")
    w1bT_d = nc.dram_tensor("w1bT", (EMB, HID), F32, kind="ExternalInput")
    w2T_d = nc.dram_tensor("w2T", (HID, HID), F32, kind="ExternalInput")
    w3v_d = nc.dram_tensor("w3v", (HID, 1024), F32, kind="ExternalInput")
    b1_d = nc.dram_tensor("b1", (HID,), F32, kind="ExternalInput")
    b2_d = nc.dram_tensor("b2", (HID,), F32, kind="ExternalInput")
    out_d = nc.dram_tensor("out", (SH, N), F32, kind="ExternalOutput")

    with tile.TileContext(nc) as tc:
        _body(tc, out_d, z1Tq_d, z2T_d, w1aT_d, w1bT_d, w2T_d, w3v_d,
              b1_d, b2_d, b3val)

    nc.compile()
    return nc


def _body(tc, out_d, z1Tq_d, z2T_d, w1aT_d, w1bT_d, w2T_d, w3v_d,
          b1_d, b2_d, b3val):
    nc = tc.nc
    with (
        tc.tile_pool(name="const", bufs=1) as const,
        tc.tile_pool(name="h1p", bufs=4) as h1p,
        tc.tile_pool(name="h2p", bufs=3) as h2p,
        tc.tile_pool(name="srows", bufs=1) as srows,
        tc.tile_pool(name="ringp", bufs=1, space="PSUM") as ringp,
    ):
        # ACT warms the gelu table as its very first instruction (no DMAs
        # ride the scalar queue at startup).
        tiny = const.tile([1, 1], F32)
        nc.vector.memset(tiny, 0.0)
        warm = const.tile([1, 1], F32)
        nc.scalar.activation(warm, tiny, GELU)

        # ---- input DMAs: z1T quarters lead the HWDGE queues ----
        z1T_sb = const.tile([128, N], FP16)
        for q, eng in enumerate((nc.sync, nc.scalar, nc.sync, nc.scalar)):
            eng.dma_start(out=z1T_sb[:, q * 256:(q + 1) * 256], in_=z1Tq_d[q].ap())
        w1aT_sb = const.tile([128, HID], FP16)
        nc.gpsimd.dma_start(out=w1aT_sb, in_=w1aT_d.ap())
        w1bT_sb = const.tile([128, HID], F32)
        nc.gpsimd.dma_start(out=w1bT_sb, in_=w1bT_d.ap())
        z2T_sb = const.tile([128, SH], F32)
        nc.sync.dma_start(out=z2T_sb, in_=z2T_d.ap())
        w2T_f = const.tile([128, HID], F32)
        nc.scalar.dma_start(out=w2T_f, in_=w2T_d.ap())
        w3v_f = const.tile([128, 1024], F32)
        nc.gpsimd.dma_start(out=w3v_f, in_=w3v_d.ap())
        b1_sb = const.tile([128, 1], F32)
        nc.gpsimd.dma_start(out=b1_sb, in_=b1_d.ap().rearrange("(p o) -> p o", o=1))
        b2_sb = const.tile([128, 1], F32)
        nc.gpsimd.dma_start(out=b2_sb, in_=b2_d.ap().rearrange("(p o) -> p o", o=1))

        c3p_sb = const.tile([128, 1], F32)
        nc.gpsimd.memset(c3p_sb, CT3)

        # fp16 stationaries (w2T on the startup-idle ACT, w3v on Pool)
        w2T_full = const.tile([128, HID], FP16)
        nc.scalar.activation(w2T_full, w2T_f, COPY, bias=0.0)
        w2T_half = const.tile([128, HID], FP16)  # x(0.5/SQB): h1' = SQB*2*gelu
        nc.scalar.activation(w2T_half, w2T_f, COPY, bias=0.0, scale=0.5 / SQB)
        w3v_h = const.tile([128, 1024], FP16)
        nc.gpsimd.tensor_copy(w3v_h, w3v_f)

        # ---- PSUM: 3 v-slots + [128,1024] W3 accumulation block ----
        ring = ringp.tile([128, 4096], F32)
        VS = [ring[:, 0:1024], ring[:, 1024:2048], ring[:, 2048:3072]]
        w3blk = ring[:, 3072:4096]

        # ---- prologue: b_pp tiles, scaled duplicated a ----
        tpb = ring[:, 2048:2048 + SH]   # v-slot 2 region, freed before use
        nc.tensor.matmul(tpb, w1bT_sb, z2T_sb)
        b_pp_sc = const.tile([128, SH], F32)       # SQB*(b + b1)
        nc.vector.tensor_scalar(out=b_pp_sc, in0=tpb, scalar1=b1_sb[:, 0:1],
                                scalar2=SQB, op0=mybir.AluOpType.add,
                                op1=mybir.AluOpType.mult)
        b_pp = const.tile([128, SH], F32)          # b + b1 (ACT slots)
        nc.vector.tensor_scalar(out=b_pp, in0=tpb, scalar1=b1_sb[:, 0:1],
                                scalar2=None, op0=mybir.AluOpType.add)
        d_sc = const.tile([128, SH], F32)          # pair bias deltas
        nc.vector.tensor_tensor(out=d_sc[:, 0:SH - 1], in0=b_pp_sc[:, 1:SH],
                                in1=b_pp_sc[:, 0:SH - 1],
                                op=mybir.AluOpType.subtract)

        tpa = ring[:, 0:1024]
        for q in range(4):
            nc.tensor.matmul(tpa[:, q * 256:(q + 1) * 256], w1aT_sb,
                             z1T_sb[:, q * 256:(q + 1) * 256])
        a_dbl = const.tile([128, 2048], F32)       # SQB*a, twice
        nc.vector.tensor_scalar(out=a_dbl[:, 0:1024], in0=tpa, scalar1=SQB,
                                scalar2=None, op0=mybir.AluOpType.mult)
        nc.scalar.activation(a_dbl[:, 1024:2048], tpa, COPY, bias=0.0,
                             scale=SQB)

        # ---- steady state ----
        srow = srows.tile([128, N], F32)
        h1map = {}

        def pump_g1(upto):
            s = pump_g1.next
            while s < min(upto, SH):
                if s in ACT_SLOTS:
                    h1 = h1p.tile([128, N], FP16, tag="h1s", name="h1s", bufs=3)
                    nc.scalar.activation(h1, a_dbl[:, 0:1024], GELU,
                                         bias=b_pp[:, s:s + 1], scale=1.0 / SQB)
                    h1map[s] = (h1, 0)
                    s += 1
                elif s + 1 < SH and (s + 1) not in ACT_SLOTS:
                    h1 = h1p.tile([128, 2048], FP16, tag="h1d", name="h1d",
                                  bufs=8)
                    nc.vector._custom_dve(
                        GELU1P_OP,
                        out=h1[:, :].rearrange("p (s n) -> p s n", n=N),
                        in0=a_dbl[:, :].rearrange("p (s n) -> p s n", n=N),
                        in1=c3p_sb[:, 0:1],
                        s0=b_pp_sc[:, s:s + 1], s1=d_sc[:, s:s + 1], imm2=CT2)
                    h1map[s] = (h1, 0)
                    h1map[s + 1] = (h1, 1024)
                    s += 2
                else:
                    h1 = h1p.tile([128, N], FP16, tag="h1s", name="h1s", bufs=3)
                    nc.vector._custom_dve(
                        GELU1_OP, out=h1, in0=a_dbl[:, 0:1024],
                        in1=c3p_sb[:, 0:1],
                        s0=b_pp_sc[:, s:s + 1], s1=1.0, imm2=CT2)
                    h1map[s] = (h1, 0)
                    s += 1
            pump_g1.next = s

        pump_g1.next = 0

        def emit_w2(s):
            h1, off = h1map.pop(s)
            w2 = w2T_full if s in ACT_SLOTS else w2T_half
            vs = VS[s % 3]
            for h in range(2):
                nc.tensor.matmul(vs[:, h * 512:(h + 1) * 512], w2,
                                 h1[:, off + h * 512:off + (h + 1) * 512])

        def emit_w3(s, h2, off):
            g, k = divmod(s, 32)
            w3k = w3v_h[:, 32 * k:32 * k + 32]
            for h in range(2):
                nc.tensor.matmul(
                    w3blk[32 * g:32 * g + 32, h * 512:(h + 1) * 512],
                    w3k, h2[:, off + h * 512:off + (h + 1) * 512],
                    tile_position=(0, 32 * g),
                    start=(k == 0), stop=(k == 31), skip_group_check=True)

        h2q = []

        def emit_g2_pair(s0, s1):
            h2 = h2p.tile([128, 2048], FP16, tag="h2", name="h2", bufs=6)
            nc.scalar.activation(h2, ring[:, (s0 % 3) * 1024:(s0 % 3) * 1024 + 2048],
                                 GELU, bias=b2_sb[:, 0:1])
            h2q.append((s0, h2, 0))
            h2q.append((s1, h2, 1024))

        def emit_g2_single(s):
            h2 = h2p.tile([128, 1024], FP16, tag="h2s", name="h2s", bufs=5)
            nc.scalar.activation(h2, VS[s % 3], GELU, bias=b2_sb[:, 0:1])
            h2q.append((s, h2, 0))

        def evac(g):
            # g==2: groups 0-2 in one pass (partition count is free);
            # g==3: final group, lands in the drain tail.
            if g < 2:
                return
            lo, hi = (0, 96) if g == 2 else (96, 128)
            nc.vector.tensor_scalar(
                out=srow[lo:hi, :], in0=w3blk[lo:hi, :],
                scalar1=b3val, scalar2=None, op0=mybir.AluOpType.add)
            nc.sync.dma_start(out=out_d.ap()[lo:hi, :], in_=srow[lo:hi, :])

        next_evac = 0
        w3_done = -1
        for s in range(SH):
            pump_g1(s + 8)
            emit_w2(s)
            r = s % 3
            if r == 1:
                emit_g2_pair(s - 1, s)
            elif r == 2:
                emit_g2_single(s)
            # W3 batched after the W2 legs of the next pair (see docstring)
            if r == 1:
                while h2q and h2q[0][0] <= s - 3:
                    sl, h2, off = h2q.pop(0)
                    emit_w3(sl, h2, off)
                    w3_done = sl
            while w3_done >= 32 * next_evac + 31:
                evac(next_evac)
                next_evac += 1
        while h2q:
            sl, h2, off = h2q.pop(0)
            emit_w3(sl, h2, off)
            w3_done = sl
            while w3_done >= 32 * next_evac + 31:
                evac(next_evac)
                next_evac += 1


_NC_CACHE = {}


def make_in_maps(z1, z2, W1, b1, W2, b2, W3, b3):
    f = np.float32
    z1 = np.asarray(z1, dtype=f)
    z2 = np.asarray(z2, dtype=f)
    W1 = np.asarray(W1, dtype=f)
    b1 = np.ascontiguousarray(np.asarray(b1, dtype=f))
    W2 = np.asarray(W2, dtype=f)
    b2 = np.ascontiguousarray(np.asarray(b2, dtype=f))
    W3 = np.asarray(W3, dtype=f)
    b3 = np.ascontiguousarray(np.asarray(b3, dtype=f))

    z1T = np.ascontiguousarray(z1.T.astype(np.float16))
    z1Tq = {
        f"z1Tq{q}": np.ascontiguousarray(z1T[:, q * 256:(q + 1) * 256])
        for q in range(4)
    }
    w1aT = np.ascontiguousarray(W1[:, :EMB].T.astype(np.float16))
    w1bT = np.ascontiguousarray(W1[:, EMB:].T)
    w2T = np.ascontiguousarray(W2.T)
    w3v = np.zeros((HID, 1024), dtype=f)
    for k in range(32):
        w3v[:, 32 * k + k] = W3[0]

    # slot permutation: widest-range i's (per core) -> ACT_SLOTS
    a = z1 @ W1[:, :EMB].T            # host copy, scheduling only
    bb = z2 @ W1[:, EMB:].T + b1
    amin, amax = a.min(0), a.max(0)
    ximax = np.maximum(np.abs(amin[None, :] + bb), np.abs(amax[None, :] + bb)).max(1)

    maps, perms = [], []
    act_slots = list(ACT_SLOTS)
    dve_slots = list(DVE_SLOTS)
    for c in range(NCORES):
        loc = ximax[c * SH:(c + 1) * SH]
        order = np.argsort(-loc)
        perm = np.empty(SH, dtype=np.int64)
        perm[act_slots] = order[:len(act_slots)]
        perm[dve_slots] = order[len(act_slots):]
        perms.append(perm)
        z2c = z2[c * SH:(c + 1) * SH][perm]
        maps.append({
            **z1Tq,
            "z2T": np.ascontiguousarray(z2c.T),
            "w1aT": w1aT, "w1bT": w1bT, "w2T": w2T, "w3v": w3v,
            "b1": b1, "b2": b2,
        })
    return maps, perms


def gather_out(results, perms):
    blocks = []
    for c in range(NCORES):
        o = results[c]["out"]
        inv = np.empty(SH, dtype=np.int64)
        inv[perms[c]] = np.arange(SH)
        blocks.append(o[inv])
    return np.concatenate(blocks, axis=0)


def kernel(z1, z2, W1, b1, W2, b2, W3, b3):
    b3v = float(np.asarray(b3).reshape(-1)[0])
    key = round(b3v, 9)
    if key not in _NC_CACHE:
        _NC_CACHE[key] = _build(b3v)
    nc = _NC_CACHE[key]

    in_maps, perms = make_in_maps(z1, z2, W1, b1, W2, b2, W3, b3)
    res = bass_utils.run_bass_kernel_spmd(nc, in_maps, core_ids=list(range(NCORES)))
    return gather_out(res.results, perms)


if __name__ == "__main__":
    rng = np.random.default_rng(0)
    s1 = 1.0 / np.sqrt(2 * EMB)
    s2 = 1.0 / np.sqrt(HID)
    ins = dict(
        z1=rng.standard_normal((N, EMB), dtype=np.float32),
        z2=rng.standard_normal((N, EMB), dtype=np.float32),
        W1=rng.uniform(-s1, s1, (HID, 2 * EMB)).astype(np.float32),
        b1=rng.uniform(-s1, s1, (HID,)).astype(np.float32),
        W2=rng.uniform(-s2, s2, (HID, HID)).astype(np.float32),
        b2=rng.uniform(-s2, s2, (HID,)).astype(np.float32),
        W3=rng.uniform(-s2, s2, (1, HID)).astype(np.float32),
        b3=rng.uniform(-s2, s2, (1,)).astype(np.float32),
    )
    out = kernel(**ins)
    print("out", out.shape, out.dtype, out[:2, :4])



# revision 20
# speedup vs baseline: 1.1423x; 1.0053x over previous
"""Fused pairwise-MLP kernel for Trainium2 (8 NeuronCores, SPMD data-parallel).

Computes log_q[i, j] = W3 @ gelu(W2 @ gelu(a[j] + b[i] + b1) + b2) + b3
with a = z1 @ W1a.T, b = z2 @ W1b.T  (W1 = [W1a | W1b]), N=1024, H=EMB=128.

Sharding: rows of i (z2) split across 8 cores, z1 + weights replicated
(host-side sharding; no collectives).

The two gelu passes (2 x 131072 128-partition columns per core) are the
arithmetic bottleneck: the ACT engine runs them at 1 elem/lane/cycle and
nothing else on the chip has a gelu table.  gelu1 therefore runs almost
entirely on the Vector engine as a runtime-registered custom DVE op that
evaluates, in one 8-ALU-stage pass over two slots (2048 cols, subdim
form, per-slot bias via PageIdx),
    y = x' + ((x'^2 + ct2)*x'^2 + ct3)*x'^2,   x' = sqrt(beta)*(a + bias)
which equals sqrt(beta)*2*gelu(x) for a beta-normalized deg-6 even fit
(the leading Horner coefficient is normalized to 1 so the three scalar
ports cover bias-step + two coefficients).  The W2 stationary for these
slots is pre-scaled by 0.5/sqrt(beta).  The NA widest-range slots per
core run on ACT's exact gelu (host permutes i-rows so they land on the
fixed ACT slot positions; permutation undone on output gather).

gelu2 runs entirely on ACT (PSUM input, b2 via the bias port) as
2048-wide pair instructions + a 1024 single per 3-slot PSUM v-ring
period.  W3 matmuls are emitted 3+ slots late and batched after the
next period's W2 legs so the in-order PE stream never head-of-line
blocks the v-ring round-trip (pair-gelu2 -> 2x W2 -> next pair).

The W3 dot uses 32 zero-padded stationary variants (w3 at column k) so
slot s = 32g+k lands at PSUM partition s of a single [128,1024]
accumulation block (tile_position group g, accumulating matmuls).  All
128 output rows are evacuated with 4 instructions (+b3) and 5 DMAs.
"""

import numpy as np

import concourse.bacc as bacc
import concourse.bass as bass
import concourse.tile as tile
import concourse.mybir as mybir
from concourse import bass_utils

import concourse.dve_ops as dve_ops
from concourse.dve_ops import DveOp, OPS
from concourse.dve_spec import (
    Spec, Src0, C0, C1, C2, C3, PageIdx, lower, _spill_c3_to_src1,
)
from concourse.dve_uop import DveOpSpec


def _register(name, spec, subdim):
    if name in dve_ops._SUB_OPCODE_FOR_NAME:
        return next(o for o in OPS if o.name == name)
    row = dve_ops._CUSTOM_DVE_ROW_BASE + len(OPS)
    dve_ops._SUB_OPCODE_FOR_NAME[name] = row
    shas = {}
    for ver in ("v3", "v4"):
        try:
            s = DveOpSpec(name=name, opcode=row, uops=lower(spec, ver=ver),
                          rd1_en=True)
            shas[ver] = s.sha(ver)
        except Exception:
            pass
    op = DveOp(name, spec, subdim=subdim, uops_sha=shas)
    OPS.append(op)
    dve_ops.CUSTOM_DVE_SPECS[name] = spec
    return op


def _gelu1_single_spec():
    # x = in0 + s0; u = x*x; out = ((s1*u + imm2)*u + c3)*u + x
    x = Src0 + C0
    u = x * x
    t = ((C1 * u + C2) * u + C3) * u
    body = _spill_c3_to_src1(t + x)

    def ref(in0, in1, s0, s1, imm2):
        xx = in0.astype(np.float32) + s0
        uu = xx * xx
        return ((s1 * uu + imm2) * uu + in1) * uu + xx

    return Spec(body=body, reference=ref)


def _gelu1_pair_spec():
    # in0 = [P, 2, N]; bias steps via PageIdx(C0, C1); leading coef = 1
    pg = PageIdx(C0, C1)
    x = Src0 + pg
    u = x * x
    t = ((u + C2) * u + C3) * u
    body = _spill_c3_to_src1(t + x)

    def ref(in0, in1, s0, s1, imm2):
        x = in0.astype(np.float32)
        S = int(np.prod(x.shape[1:-1]))
        x3 = x.reshape((x.shape[0], S, x.shape[-1]))
        idx = s0[:, None] if isinstance(s0, np.ndarray) else s0
        s1v = s1[:, None] if isinstance(s1, np.ndarray) else s1
        bias = idx + np.arange(S, dtype=np.float32)[None, :, None] * s1v
        xx = x3 + bias
        uu = xx * xx
        c3v = in1 if not isinstance(in1, np.ndarray) else in1.reshape(-1, 1, 1)
        y = ((uu + imm2) * uu + c3v) * uu + xx
        return y.reshape(in0.shape)

    return Spec(body=body, reference=ref)


GELU1_OP = _register("GELU1_EVEN6_ANT", _gelu1_single_spec(), subdim=False)
GELU1P_OP = _register("GELU1_PAIR_ANT", _gelu1_pair_spec(), subdim=True)

# deg-6 even fit of x*erf(x/sqrt(2)) on |x| <= 3.67 (density-weighted,
# x ~ N(0, 0.586)): coefs of u, u^2, u^3
CC = (0.7720335236204651, -0.09365603610221726, 0.00457457167839083)
BETA = CC[2] ** 0.4               # leading-coef normalization
SQB = float(np.sqrt(BETA))
CT2 = float(CC[1] / BETA ** 1.5)  # u'^2 coef after normalization
CT3 = float(CC[0] / SQB)          # u'^1 coef

N = 1024
EMB = 128
HID = 128
NCORES = 8
SH = N // NCORES  # i-slots per core
F32 = mybir.dt.float32
FP16 = mybir.dt.float16
GELU = mybir.ActivationFunctionType.Gelu
COPY = mybir.ActivationFunctionType.Copy

# slots whose gelu1 runs on ACT (exact); host routes widest-range i's
# here.  Gaps of 15 keep every DVE run even-length (pairable).
ACT_SLOTS = tuple(range(6, 112, 15))  # 8 slots: 6,21,...,111
DVE_SLOTS = tuple(s for s in range(SH) if s not in ACT_SLOTS)


def _build(b3val):
    nc = bacc.Bacc("TRN2", target_bir_lowering=False, debug=False)

    z1Tq_d = [
        nc.dram_tensor(f"z1Tq{q}", (EMB, 256), FP16, kind="ExternalInput")
        for q in range(4)
    ]
    z2T_d = nc.dram_tensor("z2T", (EMB, SH), F32, kind="ExternalInput")
    w1aT_d = nc.dram_tensor("w1aT", (EMB, HID), FP16, kind="ExternalInput")
    w1bT_d = nc.dram_tensor("w1bT", (EMB, HID), F32, kind="ExternalInput")
    w2T_d = nc.dram_tensor("w2T", (HID, HID), F32, kind="ExternalInput")
    w3v_d = nc.dram_tensor("w3v", (HID, 1024), F32, kind="ExternalInput")
    b1_d = nc.dram_tensor("b1", (HID,), F32, kind="ExternalInput")
    b2_d = nc.dram_tensor("b2", (HID,), F32, kind="ExternalInput")
    out_d = nc.dram_tensor("out", (SH, N), F32, kind="ExternalOutput")

    with tile.TileContext(nc) as tc:
        _body(tc, out_d, z1Tq_d, z2T_d, w1aT_d, w1bT_d, w2T_d, w3v_d,
              b1_d, b2_d, b3val)

    nc.compile()
    return nc


def _body(tc, out_d, z1Tq_d, z2T_d, w1aT_d, w1bT_d, w2T_d, w3v_d,
          b1_d, b2_d, b3val):
    nc = tc.nc
    with (
        tc.tile_pool(name="const", bufs=1) as const,
        tc.tile_pool(name="h1p", bufs=4) as h1p,
        tc.tile_pool(name="h2p", bufs=3) as h2p,
        tc.tile_pool(name="srows", bufs=1) as srows,
        tc.tile_pool(name="ringp", bufs=1, space="PSUM") as ringp,
    ):
        # ACT warms the gelu table as its very first instruction (no DMAs
        # ride the scalar queue at startup).
        tiny = const.tile([1, 1], F32)
        nc.vector.memset(tiny, 0.0)
        warm = const.tile([1, 1], F32)
        nc.scalar.activation(warm, tiny, GELU)

        # ---- input DMAs: z1T quarters lead the HWDGE queues ----
        z1T_sb = const.tile([128, N], FP16)
        for q, eng in enumerate((nc.sync, nc.scalar, nc.sync, nc.scalar)):
            eng.dma_start(out=z1T_sb[:, q * 256:(q + 1) * 256], in_=z1Tq_d[q].ap())
        w1aT_sb = const.tile([128, HID], FP16)
        nc.gpsimd.dma_start(out=w1aT_sb, in_=w1aT_d.ap())
        w1bT_sb = const.tile([128, HID], F32)
        nc.gpsimd.dma_start(out=w1bT_sb, in_=w1bT_d.ap())
        z2T_sb = const.tile([128, SH], F32)
        nc.sync.dma_start(out=z2T_sb, in_=z2T_d.ap())
        w3v_f = const.tile([128, 1024], F32)
        nc.gpsimd.dma_start(out=w3v_f, in_=w3v_d.ap())
        b1_sb = const.tile([128, 1], F32)
        nc.gpsimd.dma_start(out=b1_sb, in_=b1_d.ap().rearrange("(p o) -> p o", o=1))
        b2_sb = const.tile([128, 1], F32)
        nc.gpsimd.dma_start(out=b2_sb, in_=b2_d.ap().rearrange("(p o) -> p o", o=1))

        c3p_sb = const.tile([128, 1], F32)
        nc.gpsimd.memset(c3p_sb, CT3)


        # ---- PSUM: 3 v-slots + [128,1024] W3 accumulation block ----
        ring = ringp.tile([128, 4096], F32)
        VS = [ring[:, 0:1024], ring[:, 1024:2048], ring[:, 2048:3072]]
        w3blk = ring[:, 3072:4096]

        # ---- prologue: b_pp tiles, scaled duplicated a ----
        tpb = ring[:, 2048:2048 + SH]   # v-slot 2 region, freed before use
        nc.tensor.matmul(tpb, w1bT_sb, z2T_sb)
        b_pp_sc = const.tile([128, SH], F32)       # SQB*(b + b1)
        nc.vector.tensor_scalar(out=b_pp_sc, in0=tpb, scalar1=b1_sb[:, 0:1],
                                scalar2=SQB, op0=mybir.AluOpType.add,
                                op1=mybir.AluOpType.mult)
        b_pp = const.tile([128, SH], F32)          # b + b1 (ACT slots)
        nc.vector.tensor_scalar(out=b_pp, in0=tpb, scalar1=b1_sb[:, 0:1],
                                scalar2=None, op0=mybir.AluOpType.add)
        d_sc = const.tile([128, SH], F32)          # pair bias deltas
        nc.vector.tensor_tensor(out=d_sc[:, 0:SH - 1], in0=b_pp_sc[:, 1:SH],
                                in1=b_pp_sc[:, 0:SH - 1],
                                op=mybir.AluOpType.subtract)

        tpa = ring[:, 0:1024]
        for q in range(4):
            nc.tensor.matmul(tpa[:, q * 256:(q + 1) * 256], w1aT_sb,
                             z1T_sb[:, q * 256:(q + 1) * 256])
        a_dbl = const.tile([128, 2048], F32)       # SQB*a, twice
        nc.vector.tensor_scalar(out=a_dbl[:, 0:1024], in0=tpa, scalar1=SQB,
                                scalar2=None, op0=mybir.AluOpType.mult)
        nc.scalar.activation(a_dbl[:, 1024:2048], tpa, COPY, bias=0.0,
                             scale=SQB)

        # w2T arrives + casts happen only after the gelu1 pipeline is primed
        w2T_f = const.tile([128, HID], F32)
        nc.scalar.dma_start(out=w2T_f, in_=w2T_d.ap())
        w2T_full = const.tile([128, HID], FP16)
        nc.scalar.activation(w2T_full, w2T_f, COPY, bias=0.0)
        w2T_half = const.tile([128, HID], FP16)  # x(0.5/SQB): h1' = SQB*2*gelu
        nc.scalar.activation(w2T_half, w2T_f, COPY, bias=0.0, scale=0.5 / SQB)
        w3v_h = const.tile([128, 1024], FP16)
        nc.gpsimd.tensor_copy(w3v_h, w3v_f)

        # ---- steady state ----
        srow = srows.tile([128, N], F32)
        h1map = {}

        def pump_g1(upto):
            s = pump_g1.next
            while s < min(upto, SH):
                if s in ACT_SLOTS:
                    h1 = h1p.tile([128, N], FP16, tag="h1s", name="h1s", bufs=3)
                    nc.scalar.activation(h1, a_dbl[:, 0:1024], GELU,
                                         bias=b_pp[:, s:s + 1], scale=1.0 / SQB)
                    h1map[s] = (h1, 0)
                    s += 1
                elif s + 1 < SH and (s + 1) not in ACT_SLOTS:
                    h1 = h1p.tile([128, 2048], FP16, tag="h1d", name="h1d",
                                  bufs=6)
                    nc.vector._custom_dve(
                        GELU1P_OP,
                        out=h1[:, :].rearrange("p (s n) -> p s n", n=N),
                        in0=a_dbl[:, :].rearrange("p (s n) -> p s n", n=N),
                        in1=c3p_sb[:, 0:1],
                        s0=b_pp_sc[:, s:s + 1], s1=d_sc[:, s:s + 1], imm2=CT2)
                    h1map[s] = (h1, 0)
                    h1map[s + 1] = (h1, 1024)
                    s += 2
                else:
                    h1 = h1p.tile([128, N], FP16, tag="h1s", name="h1s", bufs=3)
                    nc.vector._custom_dve(
                        GELU1_OP, out=h1, in0=a_dbl[:, 0:1024],
                        in1=c3p_sb[:, 0:1],
                        s0=b_pp_sc[:, s:s + 1], s1=1.0, imm2=CT2)
                    h1map[s] = (h1, 0)
                    s += 1
            pump_g1.next = s

        pump_g1.next = 0

        def emit_w2(s):
            h1, off = h1map.pop(s)
            w2 = w2T_full if s in ACT_SLOTS else w2T_half
            vs = VS[s % 3]
            for h in range(2):
                nc.tensor.matmul(vs[:, h * 512:(h + 1) * 512], w2,
                                 h1[:, off + h * 512:off + (h + 1) * 512])

        def emit_w3(s, h2, off):
            g, k = divmod(s, 32)
            w3k = w3v_h[:, 32 * k:32 * k + 32]
            for h in range(2):
                nc.tensor.matmul(
                    w3blk[32 * g:32 * g + 32, h * 512:(h + 1) * 512],
                    w3k, h2[:, off + h * 512:off + (h + 1) * 512],
                    tile_position=(0, 32 * g),
                    start=(k == 0), stop=(k == 31), skip_group_check=True)

        h2q = []

        def emit_g2_pair(s0, s1):
            h2 = h2p.tile([128, 2048], FP16, tag="h2", name="h2", bufs=5)
            nc.scalar.activation(h2, ring[:, (s0 % 3) * 1024:(s0 % 3) * 1024 + 2048],
                                 GELU, bias=b2_sb[:, 0:1])
            h2q.append((s0, h2, 0))
            h2q.append((s1, h2, 1024))

        def emit_g2_single(s):
            h2 = h2p.tile([128, 1024], FP16, tag="h2s", name="h2s", bufs=4)
            nc.scalar.activation(h2, VS[s % 3], GELU, bias=b2_sb[:, 0:1])
            h2q.append((s, h2, 0))

        def evac(g):
            # g==2: groups 0-2 in one pass (partition count is free);
            # g==3: final group, lands in the drain tail.
            if g < 2:
                return
            lo, hi = (0, 96) if g == 2 else (96, 128)
            if g == 2:
                nc.scalar.activation(srow[lo:hi, :], w3blk[lo:hi, :], COPY,
                                     bias=b3val)
            else:
                nc.vector.tensor_scalar(
                    out=srow[lo:hi, :], in0=w3blk[lo:hi, :],
                    scalar1=b3val, scalar2=None, op0=mybir.AluOpType.add)
            nc.sync.dma_start(out=out_d.ap()[lo:hi, :], in_=srow[lo:hi, :])

        next_evac = 0
        w3_done = -1
        for s in range(SH):
            pump_g1(s + 6)
            emit_w2(s)
            r = s % 3
            if r == 1:
                emit_g2_pair(s - 1, s)
            elif r == 2:
                emit_g2_single(s)
            # W3 batched after the W2 legs of the next pair (see docstring)
            if r == 1:
                while h2q and h2q[0][0] <= s - 3:
                    sl, h2, off = h2q.pop(0)
                    emit_w3(sl, h2, off)
                    w3_done = sl
            while w3_done >= 32 * next_evac + 31:
                evac(next_evac)
                next_evac += 1
        while h2q:
            sl, h2, off = h2q.pop(0)
            emit_w3(sl, h2, off)
            w3_done = sl
            while w3_done >= 32 * next_evac + 31:
                evac(next_evac)
                next_evac += 1


_NC_CACHE = {}


def make_in_maps(z1, z2, W1, b1, W2, b2, W3, b3):
    f = np.float32
    z1 = np.asarray(z1, dtype=f)
    z2 = np.asarray(z2, dtype=f)
    W1 = np.asarray(W1, dtype=f)
    b1 = np.ascontiguousarray(np.asarray(b1, dtype=f))
    W2 = np.asarray(W2, dtype=f)
    b2 = np.ascontiguousarray(np.asarray(b2, dtype=f))
    W3 = np.asarray(W3, dtype=f)
    b3 = np.ascontiguousarray(np.asarray(b3, dtype=f))

    z1T = np.ascontiguousarray(z1.T.astype(np.float16))
    z1Tq = {
        f"z1Tq{q}": np.ascontiguousarray(z1T[:, q * 256:(q + 1) * 256])
        for q in range(4)
    }
    w1aT = np.ascontiguousarray(W1[:, :EMB].T.astype(np.float16))
    w1bT = np.ascontiguousarray(W1[:, EMB:].T)
    w2T = np.ascontiguousarray(W2.T)
    w3v = np.zeros((HID, 1024), dtype=f)
    for k in range(32):
        w3v[:, 32 * k + k] = W3[0]

    # slot permutation: widest-range i's (per core) -> ACT_SLOTS
    a = z1 @ W1[:, :EMB].T            # host copy, scheduling only
    bb = z2 @ W1[:, EMB:].T + b1
    amin, amax = a.min(0), a.max(0)
    ximax = np.maximum(np.abs(amin[None, :] + bb), np.abs(amax[None, :] + bb)).max(1)

    maps, perms = [], []
    act_slots = list(ACT_SLOTS)
    dve_slots = list(DVE_SLOTS)
    for c in range(NCORES):
        loc = ximax[c * SH:(c + 1) * SH]
        order = np.argsort(-loc)
        perm = np.empty(SH, dtype=np.int64)
        perm[act_slots] = order[:len(act_slots)]
        perm[dve_slots] = order[len(act_slots):]
        perms.append(perm)
        z2c = z2[c * SH:(c + 1) * SH][perm]
        maps.append({
            **z1Tq,
            "z2T": np.ascontiguousarray(z2c.T),
            "w1aT": w1aT, "w1bT": w1bT, "w2T": w2T, "w3v": w3v,
            "b1": b1, "b2": b2,
        })
    return maps, perms


def gather_out(results, perms):
    blocks = []
    for c in range(NCORES):
        o = results[c]["out"]
        inv = np.empty(SH, dtype=np.int64)
        inv[perms[c]] = np.arange(SH)
        blocks.append(o[inv])
    return np.concatenate(blocks, axis=0)


def kernel(z1, z2, W1, b1, W2, b2, W3, b3):
    b3v = float(np.asarray(b3).reshape(-1)[0])
    key = round(b3v, 9)
    if key not in _NC_CACHE:
        _NC_CACHE[key] = _build(b3v)
    nc = _NC_CACHE[key]

    in_maps, perms = make_in_maps(z1, z2, W1, b1, W2, b2, W3, b3)
    res = bass_utils.run_bass_kernel_spmd(nc, in_maps, core_ids=list(range(NCORES)))
    return gather_out(res.results, perms)


if __name__ == "__main__":
    rng = np.random.default_rng(0)
    s1 = 1.0 / np.sqrt(2 * EMB)
    s2 = 1.0 / np.sqrt(HID)
    ins = dict(
        z1=rng.standard_normal((N, EMB), dtype=np.float32),
        z2=rng.standard_normal((N, EMB), dtype=np.float32),
        W1=rng.uniform(-s1, s1, (HID, 2 * EMB)).astype(np.float32),
        b1=rng.uniform(-s1, s1, (HID,)).astype(np.float32),
        W2=rng.uniform(-s2, s2, (HID, HID)).astype(np.float32),
        b2=rng.uniform(-s2, s2, (HID,)).astype(np.float32),
        W3=rng.uniform(-s2, s2, (1, HID)).astype(np.float32),
        b3=rng.uniform(-s2, s2, (1,)).astype(np.float32),
    )
    out = kernel(**ins)
    print("out", out.shape, out.dtype, out[:2, :4])



# revision 21
# speedup vs baseline: 1.1450x; 1.0023x over previous
"""Fused pairwise-MLP kernel for Trainium2 (8 NeuronCores, SPMD data-parallel).

Computes log_q[i, j] = W3 @ gelu(W2 @ gelu(a[j] + b[i] + b1) + b2) + b3
with a = z1 @ W1a.T, b = z2 @ W1b.T  (W1 = [W1a | W1b]), N=1024, H=EMB=128.

Sharding: rows of i (z2) split across 8 cores, z1 + weights replicated
(host-side sharding; no collectives).

The two gelu passes (2 x 131072 128-partition columns per core) are the
arithmetic bottleneck: the ACT engine runs them at 1 elem/lane/cycle and
nothing else on the chip has a gelu table.  gelu1 therefore runs almost
entirely on the Vector engine as a runtime-registered custom DVE op that
evaluates, in one 8-ALU-stage pass over two slots (2048 cols, subdim
form, per-slot bias via PageIdx),
    y = x' + ((x'^2 + ct2)*x'^2 + ct3)*x'^2,   x' = sqrt(beta)*(a + bias)
which equals sqrt(beta)*2*gelu(x) for a beta-normalized deg-6 even fit
(the leading Horner coefficient is normalized to 1 so the three scalar
ports cover bias-step + two coefficients).  The W2 stationary for these
slots is pre-scaled by 0.5/sqrt(beta).  The NA widest-range slots per
core run on ACT's exact gelu (host permutes i-rows so they land on the
fixed ACT slot positions; permutation undone on output gather).

gelu2 runs entirely on ACT (PSUM input, b2 via the bias port) as
2048-wide pair instructions + a 1024 single per 3-slot PSUM v-ring
period.  W3 matmuls are emitted 3+ slots late and batched after the
next period's W2 legs so the in-order PE stream never head-of-line
blocks the v-ring round-trip (pair-gelu2 -> 2x W2 -> next pair).

The W3 dot uses 32 zero-padded stationary variants (w3 at column k) so
slot s = 32g+k lands at PSUM partition s of a single [128,1024]
accumulation block (tile_position group g, accumulating matmuls).  All
128 output rows are evacuated with 4 instructions (+b3) and 5 DMAs.
"""

import numpy as np

import concourse.bacc as bacc
import concourse.bass as bass
import concourse.tile as tile
import concourse.mybir as mybir
from concourse import bass_utils

import concourse.dve_ops as dve_ops
from concourse.dve_ops import DveOp, OPS
from concourse.dve_spec import (
    Spec, Src0, C0, C1, C2, C3, PageIdx, lower, _spill_c3_to_src1,
)
from concourse.dve_uop import DveOpSpec


def _register(name, spec, subdim):
    if name in dve_ops._SUB_OPCODE_FOR_NAME:
        return next(o for o in OPS if o.name == name)
    row = dve_ops._CUSTOM_DVE_ROW_BASE + len(OPS)
    dve_ops._SUB_OPCODE_FOR_NAME[name] = row
    shas = {}
    for ver in ("v3", "v4"):
        try:
            s = DveOpSpec(name=name, opcode=row, uops=lower(spec, ver=ver),
                          rd1_en=True)
            shas[ver] = s.sha(ver)
        except Exception:
            pass
    op = DveOp(name, spec, subdim=subdim, uops_sha=shas)
    OPS.append(op)
    dve_ops.CUSTOM_DVE_SPECS[name] = spec
    return op


def _gelu1_single_spec():
    # x = in0 + s0; u = x*x; out = ((s1*u + imm2)*u + c3)*u + x
    x = Src0 + C0
    u = x * x
    t = ((C1 * u + C2) * u + C3) * u
    body = _spill_c3_to_src1(t + x)

    def ref(in0, in1, s0, s1, imm2):
        xx = in0.astype(np.float32) + s0
        uu = xx * xx
        return ((s1 * uu + imm2) * uu + in1) * uu + xx

    return Spec(body=body, reference=ref)


def _gelu1_pair_spec():
    # in0 = [P, 2, N]; bias steps via PageIdx(C0, C1); leading coef = 1
    pg = PageIdx(C0, C1)
    x = Src0 + pg
    u = x * x
    t = ((u + C2) * u + C3) * u
    body = _spill_c3_to_src1(t + x)

    def ref(in0, in1, s0, s1, imm2):
        x = in0.astype(np.float32)
        S = int(np.prod(x.shape[1:-1]))
        x3 = x.reshape((x.shape[0], S, x.shape[-1]))
        idx = s0[:, None] if isinstance(s0, np.ndarray) else s0
        s1v = s1[:, None] if isinstance(s1, np.ndarray) else s1
        bias = idx + np.arange(S, dtype=np.float32)[None, :, None] * s1v
        xx = x3 + bias
        uu = xx * xx
        c3v = in1 if not isinstance(in1, np.ndarray) else in1.reshape(-1, 1, 1)
        y = ((uu + imm2) * uu + c3v) * uu + xx
        return y.reshape(in0.shape)

    return Spec(body=body, reference=ref)


GELU1_OP = _register("GELU1_EVEN6_ANT", _gelu1_single_spec(), subdim=False)
GELU1P_OP = _register("GELU1_PAIR_ANT", _gelu1_pair_spec(), subdim=True)

# deg-6 even fit of x*erf(x/sqrt(2)) on |x| <= 3.67 (density-weighted,
# x ~ N(0, 0.586)): coefs of u, u^2, u^3
CC = (0.7720335236204651, -0.09365603610221726, 0.00457457167839083)
BETA = CC[2] ** 0.4               # leading-coef normalization
SQB = float(np.sqrt(BETA))
CT2 = float(CC[1] / BETA ** 1.5)  # u'^2 coef after normalization
CT3 = float(CC[0] / SQB)          # u'^1 coef

N = 1024
EMB = 128
HID = 128
NCORES = 8
SH = N // NCORES  # i-slots per core
F32 = mybir.dt.float32
FP16 = mybir.dt.float16
GELU = mybir.ActivationFunctionType.Gelu
COPY = mybir.ActivationFunctionType.Copy

# slots whose gelu1 runs on ACT (exact); host routes widest-range i's
# here.  Gaps of 15 keep every DVE run even-length (pairable).
ACT_SLOTS = tuple(range(6, 112, 15))  # 8 slots: 6,21,...,111
DVE_SLOTS = tuple(s for s in range(SH) if s not in ACT_SLOTS)


def _build(b3val):
    nc = bacc.Bacc("TRN2", target_bir_lowering=False, debug=False)

    z1Tq_d = [
        nc.dram_tensor(f"z1Tq{q}", (EMB, 256), FP16, kind="ExternalInput")
        for q in range(4)
    ]
    z2T_d = nc.dram_tensor("z2T", (EMB, SH), F32, kind="ExternalInput")
    w1aT_d = nc.dram_tensor("w1aT", (EMB, HID), FP16, kind="ExternalInput")
    w1bT_d = nc.dram_tensor("w1bT", (EMB, HID), F32, kind="ExternalInput")
    w2T_d = nc.dram_tensor("w2T", (HID, HID), F32, kind="ExternalInput")
    w3v_d = nc.dram_tensor("w3v", (HID, 1024), F32, kind="ExternalInput")
    b1_d = nc.dram_tensor("b1", (HID,), F32, kind="ExternalInput")
    b2_d = nc.dram_tensor("b2", (HID,), F32, kind="ExternalInput")
    out_d = nc.dram_tensor("out", (SH, N), F32, kind="ExternalOutput")

    with tile.TileContext(nc) as tc:
        _body(tc, out_d, z1Tq_d, z2T_d, w1aT_d, w1bT_d, w2T_d, w3v_d,
              b1_d, b2_d, b3val)

    nc.compile()
    return nc


def _body(tc, out_d, z1Tq_d, z2T_d, w1aT_d, w1bT_d, w2T_d, w3v_d,
          b1_d, b2_d, b3val):
    nc = tc.nc
    with (
        tc.tile_pool(name="const", bufs=1) as const,
        tc.tile_pool(name="h1p", bufs=4) as h1p,
        tc.tile_pool(name="h2p", bufs=3) as h2p,
        tc.tile_pool(name="srows", bufs=1) as srows,
        tc.tile_pool(name="ringp", bufs=1, space="PSUM") as ringp,
    ):
        # ACT warms the gelu table as its very first instruction (no DMAs
        # ride the scalar queue at startup).
        tiny = const.tile([1, 1], F32)
        nc.vector.memset(tiny, 0.0)
        warm = const.tile([1, 1], F32)
        nc.scalar.activation(warm, tiny, GELU)

        # ---- input DMAs: z1T quarters lead the HWDGE queues ----
        w1aT_sb = const.tile([128, HID], FP16)
        nc.gpsimd.dma_start(out=w1aT_sb, in_=w1aT_d.ap())
        w1bT_sb = const.tile([128, HID], F32)
        nc.gpsimd.dma_start(out=w1bT_sb, in_=w1bT_d.ap())
        z1T_sb = const.tile([128, N], FP16)
        for q, eng in enumerate((nc.sync, nc.gpsimd, nc.sync, nc.gpsimd)):
            eng.dma_start(out=z1T_sb[:, q * 256:(q + 1) * 256], in_=z1Tq_d[q].ap())
        z2T_sb = const.tile([128, SH], F32)
        nc.sync.dma_start(out=z2T_sb, in_=z2T_d.ap())
        w3v_f = const.tile([128, 1024], F32)
        nc.gpsimd.dma_start(out=w3v_f, in_=w3v_d.ap())
        b1_sb = const.tile([128, 1], F32)
        nc.gpsimd.dma_start(out=b1_sb, in_=b1_d.ap().rearrange("(p o) -> p o", o=1))
        b2_sb = const.tile([128, 1], F32)
        nc.gpsimd.dma_start(out=b2_sb, in_=b2_d.ap().rearrange("(p o) -> p o", o=1))

        c3p_sb = const.tile([128, 1], F32)
        nc.gpsimd.memset(c3p_sb, CT3)


        # ---- PSUM: 3 v-slots + [128,1024] W3 accumulation block ----
        ring = ringp.tile([128, 4096], F32)
        VS = [ring[:, 0:1024], ring[:, 1024:2048], ring[:, 2048:3072]]
        w3blk = ring[:, 3072:4096]

        # ---- prologue: b_pp tiles, scaled duplicated a ----
        tpb = ring[:, 2048:2048 + SH]   # v-slot 2 region, freed before use
        nc.tensor.matmul(tpb, w1bT_sb, z2T_sb)
        b_pp_sc = const.tile([128, SH], F32)       # SQB*(b + b1)
        nc.vector.tensor_scalar(out=b_pp_sc, in0=tpb, scalar1=b1_sb[:, 0:1],
                                scalar2=SQB, op0=mybir.AluOpType.add,
                                op1=mybir.AluOpType.mult)
        b_pp = const.tile([128, SH], F32)          # b + b1 (ACT slots)
        nc.vector.tensor_scalar(out=b_pp, in0=tpb, scalar1=b1_sb[:, 0:1],
                                scalar2=None, op0=mybir.AluOpType.add)
        d_sc = const.tile([128, SH], F32)          # pair bias deltas
        nc.vector.tensor_tensor(out=d_sc[:, 0:SH - 1], in0=b_pp_sc[:, 1:SH],
                                in1=b_pp_sc[:, 0:SH - 1],
                                op=mybir.AluOpType.subtract)

        tpa = ring[:, 0:1024]
        for q in range(4):
            nc.tensor.matmul(tpa[:, q * 256:(q + 1) * 256], w1aT_sb,
                             z1T_sb[:, q * 256:(q + 1) * 256])
        a_dbl = const.tile([128, 2048], F32)       # SQB*a, twice
        nc.vector.tensor_scalar(out=a_dbl[:, 0:1024], in0=tpa, scalar1=SQB,
                                scalar2=None, op0=mybir.AluOpType.mult)
        nc.scalar.activation(a_dbl[:, 1024:2048], tpa, COPY, bias=0.0,
                             scale=SQB)

        # w2T arrives + casts happen only after the gelu1 pipeline is primed
        w2T_f = const.tile([128, HID], F32)
        nc.scalar.dma_start(out=w2T_f, in_=w2T_d.ap())
        w2T_full = const.tile([128, HID], FP16)
        nc.scalar.activation(w2T_full, w2T_f, COPY, bias=0.0)
        w2T_half = const.tile([128, HID], FP16)  # x(0.5/SQB): h1' = SQB*2*gelu
        nc.scalar.activation(w2T_half, w2T_f, COPY, bias=0.0, scale=0.5 / SQB)
        w3v_h = const.tile([128, 1024], FP16)
        nc.gpsimd.tensor_copy(w3v_h, w3v_f)

        # ---- steady state ----
        srow = srows.tile([128, N], F32)
        h1map = {}

        def pump_g1(upto):
            s = pump_g1.next
            while s < min(upto, SH):
                if s in ACT_SLOTS:
                    h1 = h1p.tile([128, N], FP16, tag="h1s", name="h1s", bufs=3)
                    nc.scalar.activation(h1, a_dbl[:, 0:1024], GELU,
                                         bias=b_pp[:, s:s + 1], scale=1.0 / SQB)
                    h1map[s] = (h1, 0)
                    s += 1
                elif s + 1 < SH and (s + 1) not in ACT_SLOTS:
                    h1 = h1p.tile([128, 2048], FP16, tag="h1d", name="h1d",
                                  bufs=6)
                    nc.vector._custom_dve(
                        GELU1P_OP,
                        out=h1[:, :].rearrange("p (s n) -> p s n", n=N),
                        in0=a_dbl[:, :].rearrange("p (s n) -> p s n", n=N),
                        in1=c3p_sb[:, 0:1],
                        s0=b_pp_sc[:, s:s + 1], s1=d_sc[:, s:s + 1], imm2=CT2)
                    h1map[s] = (h1, 0)
                    h1map[s + 1] = (h1, 1024)
                    s += 2
                else:
                    h1 = h1p.tile([128, N], FP16, tag="h1s", name="h1s", bufs=3)
                    nc.vector._custom_dve(
                        GELU1_OP, out=h1, in0=a_dbl[:, 0:1024],
                        in1=c3p_sb[:, 0:1],
                        s0=b_pp_sc[:, s:s + 1], s1=1.0, imm2=CT2)
                    h1map[s] = (h1, 0)
                    s += 1
            pump_g1.next = s

        pump_g1.next = 0

        def emit_w2(s):
            h1, off = h1map.pop(s)
            w2 = w2T_full if s in ACT_SLOTS else w2T_half
            vs = VS[s % 3]
            for h in range(2):
                nc.tensor.matmul(vs[:, h * 512:(h + 1) * 512], w2,
                                 h1[:, off + h * 512:off + (h + 1) * 512])

        def emit_w3(s, h2, off):
            g, k = divmod(s, 32)
            w3k = w3v_h[:, 32 * k:32 * k + 32]
            for h in range(2):
                nc.tensor.matmul(
                    w3blk[32 * g:32 * g + 32, h * 512:(h + 1) * 512],
                    w3k, h2[:, off + h * 512:off + (h + 1) * 512],
                    tile_position=(0, 32 * g),
                    start=(k == 0), stop=(k == 31), skip_group_check=True)

        h2q = []

        def emit_g2_pair(s0, s1):
            h2 = h2p.tile([128, 2048], FP16, tag="h2", name="h2", bufs=5)
            nc.scalar.activation(h2, ring[:, (s0 % 3) * 1024:(s0 % 3) * 1024 + 2048],
                                 GELU, bias=b2_sb[:, 0:1])
            h2q.append((s0, h2, 0))
            h2q.append((s1, h2, 1024))

        def emit_g2_single(s):
            h2 = h2p.tile([128, 1024], FP16, tag="h2s", name="h2s", bufs=4)
            nc.scalar.activation(h2, VS[s % 3], GELU, bias=b2_sb[:, 0:1])
            h2q.append((s, h2, 0))

        def evac(g):
            # g==2: groups 0-2 in one pass (partition count is free);
            # g==3: final group, lands in the drain tail.
            if g < 2:
                return
            lo, hi = (0, 96) if g == 2 else (96, 128)
            if g == 2:
                nc.scalar.activation(srow[lo:hi, :], w3blk[lo:hi, :], COPY,
                                     bias=b3val)
            else:
                nc.vector.tensor_scalar(
                    out=srow[lo:hi, :], in0=w3blk[lo:hi, :],
                    scalar1=b3val, scalar2=None, op0=mybir.AluOpType.add)
            nc.sync.dma_start(out=out_d.ap()[lo:hi, :], in_=srow[lo:hi, :])

        next_evac = 0
        w3_done = -1
        for s in range(SH):
            pump_g1(s + 6)
            emit_w2(s)
            r = s % 3
            if r == 1:
                emit_g2_pair(s - 1, s)
            elif r == 2:
                emit_g2_single(s)
            # W3 batched after the W2 legs of the next pair (see docstring)
            if r == 1:
                while h2q and h2q[0][0] <= s - 3:
                    sl, h2, off = h2q.pop(0)
                    emit_w3(sl, h2, off)
                    w3_done = sl
            while w3_done >= 32 * next_evac + 31:
                evac(next_evac)
                next_evac += 1
        while h2q:
            sl, h2, off = h2q.pop(0)
            emit_w3(sl, h2, off)
            w3_done = sl
            while w3_done >= 32 * next_evac + 31:
                evac(next_evac)
                next_evac += 1


_NC_CACHE = {}


def make_in_maps(z1, z2, W1, b1, W2, b2, W3, b3):
    f = np.float32
    z1 = np.asarray(z1, dtype=f)
    z2 = np.asarray(z2, dtype=f)
    W1 = np.asarray(W1, dtype=f)
    b1 = np.ascontiguousarray(np.asarray(b1, dtype=f))
    W2 = np.asarray(W2, dtype=f)
    b2 = np.ascontiguousarray(np.asarray(b2, dtype=f))
    W3 = np.asarray(W3, dtype=f)
    b3 = np.ascontiguousarray(np.asarray(b3, dtype=f))

    z1T = np.ascontiguousarray(z1.T.astype(np.float16))
    z1Tq = {
        f"z1Tq{q}": np.ascontiguousarray(z1T[:, q * 256:(q + 1) * 256])
        for q in range(4)
    }
    w1aT = np.ascontiguousarray(W1[:, :EMB].T.astype(np.float16))
    w1bT = np.ascontiguousarray(W1[:, EMB:].T)
    w2T = np.ascontiguousarray(W2.T)
    w3v = np.zeros((HID, 1024), dtype=f)
    for k in range(32):
        w3v[:, 32 * k + k] = W3[0]

    # slot permutation: widest-range i's (per core) -> ACT_SLOTS
    a = z1 @ W1[:, :EMB].T            # host copy, scheduling only
    bb = z2 @ W1[:, EMB:].T + b1
    amin, amax = a.min(0), a.max(0)
    ximax = np.maximum(np.abs(amin[None, :] + bb), np.abs(amax[None, :] + bb)).max(1)

    maps, perms = [], []
    act_slots = list(ACT_SLOTS)
    dve_slots = list(DVE_SLOTS)
    for c in range(NCORES):
        loc = ximax[c * SH:(c + 1) * SH]
        order = np.argsort(-loc)
        perm = np.empty(SH, dtype=np.int64)
        perm[act_slots] = order[:len(act_slots)]
        perm[dve_slots] = order[len(act_slots):]
        perms.append(perm)
        z2c = z2[c * SH:(c + 1) * SH][perm]
        maps.append({
            **z1Tq,
            "z2T": np.ascontiguousarray(z2c.T),
            "w1aT": w1aT, "w1bT": w1bT, "w2T": w2T, "w3v": w3v,
            "b1": b1, "b2": b2,
        })
    return maps, perms


def gather_out(results, perms):
    blocks = []
    for c in range(NCORES):
        o = results[c]["out"]
        inv = np.empty(SH, dtype=np.int64)
        inv[perms[c]] = np.arange(SH)
        blocks.append(o[inv])
    return np.concatenate(blocks, axis=0)


def kernel(z1, z2, W1, b1, W2, b2, W3, b3):
    b3v = float(np.asarray(b3).reshape(-1)[0])
    key = round(b3v, 9)
    if key not in _NC_CACHE:
        _NC_CACHE[key] = _build(b3v)
    nc = _NC_CACHE[key]

    in_maps, perms = make_in_maps(z1, z2, W1, b1, W2, b2, W3, b3)
    res = bass_utils.run_bass_kernel_spmd(nc, in_maps, core_ids=list(range(NCORES)))
    return gather_out(res.results, perms)


if __name__ == "__main__":
    rng = np.random.default_rng(0)
    s1 = 1.0 / np.sqrt(2 * EMB)
    s2 = 1.0 / np.sqrt(HID)
    ins = dict(
        z1=rng.standard_normal((N, EMB), dtype=np.float32),
        z2=rng.standard_normal((N, EMB), dtype=np.float32),
        W1=rng.uniform(-s1, s1, (HID, 2 * EMB)).astype(np.float32),
        b1=rng.uniform(-s1, s1, (HID,)).astype(np.float32),
        W2=rng.uniform(-s2, s2, (HID, HID)).astype(np.float32),
        b2=rng.uniform(-s2, s2, (HID,)).astype(np.float32),
        W3=rng.uniform(-s2, s2, (1, HID)).astype(np.float32),
        b3=rng.uniform(-s2, s2, (1,)).astype(np.float32),
    )
    out = kernel(**ins)
    print("out", out.shape, out.dtype, out[:2, :4])

